# revision 28
# baseline (speedup 1.0000x reference)
"""NeuralSort relaxed-permutation kernel for 8 Trainium2 NeuronCores.

out[b, i, j] = softmax_i( s_i * scaling_j - B_i ),  s = -scores[b]
  scaling_j = n - 1 - 2j   =>  z[i,j] = c_j * x_i - B_i  with x = scores[b],
  c_j = -(n - 1 - 2j) = 2j + 1 - n
  B_i = sum_k |x_i - x_k| = x_i*(n - 2*cnt_i) - S + 2*t_i
        cnt_i = #{k: x_k > x_i},  t_i = sum_{k: x_k > x_i} x_k,  S = sum_k x_k

Sharding: core c -> (batch b = c//2, j-half h = c%2). Each core emits the
full-i (n) by half-j (n/2) slab of batch b.

Per-core pipeline (all matmuls bf16 @ 1 cyc/row, exact via hi/mid/lo splits):
  P: G[k,i] = [x_k > x_i] on DVE (tensor_scalar is_lt, bf16 out);
     PE: [ones|x_hi|x_lo]^T @ G accumulated over k-chunks -> cnt/t rows;
     combine to B row on partition 0; AllGather halves within the pair.
  S: z = c_j x_i - B_i via K=9 bf16 stacked matmul into PSUM (j on partitions,
     i on free); DVE reduce_max -> M_j; re-matmul z; ACT exp(z - M_j) with
     accum -> D_j; Q_j = M_j + ln(D_j) (ACT Log, same table set as Exp).
  O: z' = c_j x_i - Q_j via K=9 bf16 matmul (i on partitions, j on free);
     ACT exp(z' - B_i) straight from PSUM -> final values in SBUF; big DMAs.
"""

from contextlib import ExitStack

import numpy as np
import ml_dtypes

import concourse.bass as bass
import concourse.tile as tile
from concourse import bacc, mybir
from concourse.bass_utils import run_bass_kernel_spmd

F32 = mybir.dt.float32
BF16 = mybir.dt.bfloat16
AF = mybir.ActivationFunctionType
ALU = mybir.AluOpType

N_CORES = 8
P = 128


def _bf(x):
    return np.asarray(x, dtype=ml_dtypes.bfloat16)


def _split3(x):
    x = np.asarray(x, dtype=np.float32)
    h = _bf(x)
    r = x - h.astype(np.float32)
    m = _bf(r)
    l = _bf(r - m.astype(np.float32))
    return h, m, l


def _split2(x):
    x = np.asarray(x, dtype=np.float32)
    h = _bf(x)
    l = _bf(x - h.astype(np.float32))
    return h, l


# K-row pairing for the z matmuls (z = sum_k lhsT_row_k * rhs_row_k).
# Stacks put PE-transposed rows first (matmul outputs must land at partition
# base 0); DMA-filled rows follow (DMA can target any partition).
# Stats: lhsT rows from [chi, clo, ones]; rhs rows from [Bh,Bm,Bl,xh,xm,xl].
SEL_L_S = np.zeros((3, 9), np.float32)
SEL_R_S = np.zeros((6, 9), np.float32)
_PAIRS_S = [
    (0, 3, 1.0),   # c_hi * x_h
    (2, 0, -1.0),  # 1 * -B_h
    (1, 3, 1.0),   # c_lo * x_h
    (0, 4, 1.0),   # c_hi * x_m
    (2, 1, -1.0),  # 1 * -B_m
    (1, 4, 1.0),   # c_lo * x_m
    (0, 5, 1.0),   # c_hi * x_l
    (2, 2, -1.0),  # 1 * -B_l
    (1, 5, 1.0),   # c_lo * x_l
]
for k, (ls, rs, w) in enumerate(_PAIRS_S):
    SEL_L_S[ls, k] = 1.0
    SEL_R_S[rs, k] = w

# Output: lhsT rows from [xh, xm, xl, ones]; rhs rows from [Qh, Qm, Ql, chi, clo].
SEL_L_O = np.zeros((4, 9), np.float32)
SEL_R_O = np.zeros((5, 9), np.float32)
_PAIRS_O = [
    (0, 3, 1.0),   # x_h * c_hi
    (3, 0, -1.0),  # 1 * -Q_h
    (0, 4, 1.0),   # x_h * c_lo
    (3, 1, -1.0),  # 1 * -Q_m
    (1, 3, 1.0),   # x_m * c_hi
    (1, 4, 1.0),   # x_m * c_lo
    (2, 3, 1.0),   # x_l * c_hi
    (3, 2, -1.0),  # 1 * -Q_l
    (2, 4, 1.0),   # x_l * c_lo
]
for k, (ls, rs, w) in enumerate(_PAIRS_O):
    SEL_L_O[ls, k] = 1.0
    SEL_R_O[rs, k] = w


def build_nc(n=4096, mode="pair", num_devices=N_CORES):
    """mode: "pair" (8-core, AllGather B halves) or "single" (1-core debug:
    full j-range and full i-range on one core, no collective)."""
    single = mode == "single"
    nj = n if single else n // 2    # output columns per core
    nih = n if single else n // 2   # i-range whose B this core computes
    nkc = n // P                    # k-chunks in the G pass
    njc = nj // P                   # 128-wide j-chunks for stats
    nic = n // P                    # 128-wide i-chunks for output
    jg = 512                        # output j staging group width
    n_jg = nj // jg
    jcs_per_g = jg // P
    icg = min(16, nic)              # i-chunks per output DMA
    niq = max(2, n // 1024)         # stats i-quarter count
    iq = n // niq                   # stats free-dim per z tile (<=1024)

    nc = bacc.Bacc(
        "TRN2", target_bir_lowering=False, debug=False, num_devices=num_devices
    )

    def din(name, shape, dt=F32):
        return nc.dram_tensor(name, shape, dt, kind="ExternalInput").ap()

    xh = din("xh", [1, n], BF16)
    xm = din("xm", [1, n], BF16)
    xl = din("xl", [1, n], BF16)
    xhh = din("xhh", [1, nih], BF16)
    xmh = din("xmh", [1, nih], BF16)
    xlh = din("xlh", [1, nih], BF16)
    xcol = din("xcol", [P, nkc], F32)
    xhc = din("xhc", [P, nih // P], F32)  # x of this core's i-half, col layout
    blhs = din("blhs", [P, 3 * nkc], BF16)
    xc2 = din("xc2", [P, 2 * nkc], BF16)
    chi = din("chi", [1, nj], BF16)
    clo = din("clo", [1, nj], BF16)
    i128 = din("i128", [P, P], BF16)
    sel_l_s = din("sel_l_s", [3, 9], BF16)
    sel_r_s = din("sel_r_s", [6, 9], BF16)
    sel_l_o = din("sel_l_o", [4, 9], BF16)
    sel_r_o = din("sel_r_o", [5, 9], BF16)
    ones3 = din("ones3", [3, P], BF16)
    onesc = din("onesc", [P, 1], BF16)
    ones_row = din("ones_row", [1, n], BF16)

    out = nc.dram_tensor("out", [n, nj], F32, kind="ExternalOutput").ap()

    bh_dram = nc.dram_tensor("bh_dram", [1, nih], F32).ap()
    nhalves = 1 if single else 2
    bfull_dram = nc.dram_tensor("bfull_dram", [nhalves, nih], F32).ap()
    b3_dram = nc.dram_tensor("b3_dram", [3, nih], F32).ap()
    bspl_dram = nc.dram_tensor("bspl_dram", [3, n], BF16).ap()
    groups = [[2 * p, 2 * p + 1] for p in range(max(1, num_devices // 2))]

    def mm512(out_ap, lhsT, rhs, start=True, stop=True):
        """matmul with the moving dim split into <=512-column chunks."""
        nfree = rhs.shape[-1]
        assert out_ap.shape[-1] == nfree
        for o in range(0, nfree, 512):
            e = min(o + 512, nfree)
            nc.tensor.matmul(
                out_ap[..., o:e], lhsT, rhs[..., o:e], start=start, stop=stop
            )

    with tile.TileContext(nc) as tc, ExitStack() as ctx:
        cpool = ctx.enter_context(tc.tile_pool(name="consts", bufs=1))

        def load(pool, ap_dram, shape, dt, name):
            t = pool.tile(shape, dt, tag=name)
            nc.sync.dma_start(out=t[:], in_=ap_dram)
            return t

        # long-lived constants
        i128_s = load(cpool, i128, [P, P], BF16, "i128")
        sel_r_o_s = load(cpool, sel_r_o, [5, 9], BF16, "sel_r_o")
        bcol = cpool.tile([P, nic], F32, tag="bcol")
        nbcol = cpool.tile([P, nic], F32, tag="nbcol")
        l9 = cpool.tile([9, nj], BF16, tag="l9")
        r9 = [
            cpool.tile([9, nih], BF16, tag=f"r9_{h}", name=f"r9_{h}")
            for h in range(nhalves)
        ]
        l9o = cpool.tile([9, n], BF16, tag="l9o")
        r9o = cpool.tile([9, nj], BF16, tag="r9o")
        src_o = cpool.tile([5, nj], BF16, tag="src_o")
        qcol = cpool.tile([P, njc], F32, tag="qcol")

        with tc.tile_pool(name="prep", bufs=1) as pp_s:
            xcol_s = load(pp_s, xcol, [P, nkc], F32, "xcol")
            xhc_s = load(pp_s, xhc, [P, nih // P], F32, "xhc")
            blhs_s = load(pp_s, blhs, [P, 3 * nkc], BF16, "blhs")
            xc2_s = load(pp_s, xc2, [P, 2 * nkc], BF16, "xc2")
            sel_l_s_s = load(pp_s, sel_l_s, [3, 9], BF16, "sel_l_s")
            sel_r_s_s = load(pp_s, sel_r_s, [6, 9], BF16, "sel_r_s")
            sel_l_o_s = load(pp_s, sel_l_o, [4, 9], BF16, "sel_l_o")
            ones3_s = load(pp_s, ones3, [3, P], BF16, "ones3")
            onesc_s = load(pp_s, onesc, [P, 1], BF16, "onesc")
            xsh = pp_s.tile([3, nih], BF16, tag="xsh")
            nc.sync.dma_start(out=xsh[0:1, :], in_=xhh)
            nc.sync.dma_start(out=xsh[1:2, :], in_=xmh)
            nc.sync.dma_start(out=xsh[2:3, :], in_=xlh)

            # ---- Phase P: B over this core's i-half ----
            xb = pp_s.tile([P, nih], F32, tag="xb")
            with tc.tile_pool(name="pp", bufs=1, space="PSUM") as pp:
                xbp = pp.tile([P, nih], F32)
                mm512(xbp[:], ones3_s[:], xsh[:])
                nc.scalar.copy(out=xb[:], in_=xbp[:])

            b3 = pp_s.tile([3, nih], F32, tag="b3")
            with (
                tc.tile_pool(name="bp", bufs=1, space="PSUM") as bp,
                tc.tile_pool(name="gp", bufs=3) as gp,
            ):
                bpsum = bp.tile([3, nih], F32)
                for k in range(nkc):
                    g = gp.tile([P, nih], BF16, tag="g")
                    nc.vector.tensor_scalar(
                        out=g[:],
                        in0=xb[:],
                        scalar1=xcol_s[:, k : k + 1],
                        scalar2=None,
                        op0=ALU.is_lt,
                    )
                    mm512(
                        bpsum[:],
                        blhs_s[:, 3 * k : 3 * k + 3],
                        g[:],
                        start=(k == 0),
                        stop=(k == nkc - 1),
                    )
                nc.scalar.copy(out=b3[:], in_=bpsum[:])

            # roundtrip cnt/t rows through DRAM into column layout [128, nihc]
            nihc = nih // P
            nc.sync.dma_start(out=b3_dram, in_=b3[:])
            cntc = pp_s.tile([P, nihc], F32, tag="cntc")
            thc = pp_s.tile([P, nihc], F32, tag="thc")
            tlc = pp_s.tile([P, nihc], F32, tag="tlc")
            for t_, row in ((cntc, 0), (thc, 1), (tlc, 2)):
                nc.sync.dma_start(
                    out=t_[:],
                    in_=b3_dram[row].rearrange("(t p) -> p t", p=P),
                )

            sneg = pp_s.tile([1, 1], F32, tag="sneg")
            with tc.tile_pool(name="sp", bufs=1, space="PSUM") as sp:
                sxp = sp.tile([1, 2 * nkc], F32)
                nc.tensor.matmul(
                    sxp[:], onesc_s[:], xc2_s[:], start=True, stop=True
                )
                ssum = pp_s.tile([1, 1], F32, tag="ssum")
                nc.vector.tensor_reduce(
                    out=ssum[:], in_=sxp[:], axis=mybir.AxisListType.X, op=ALU.add
                )
                nc.vector.tensor_scalar_mul(sneg[:], ssum[:], -1.0)
            snegc = pp_s.tile([P, 1], F32, tag="snegc")
            nc.gpsimd.partition_broadcast(snegc[:], sneg[0:1, 0:1])

            # B = x*(n - 2*cnt) - S + 2*(th + tl), all in column layout
            r1 = pp_s.tile([P, nihc], F32, tag="r1")
            nc.vector.tensor_scalar(
                out=r1[:],
                in0=cntc[:],
                scalar1=-2.0,
                scalar2=float(n),
                op0=ALU.mult,
                op1=ALU.add,
            )
            r2 = pp_s.tile([P, nihc], F32, tag="r2")
            nc.vector.tensor_tensor(out=r2[:], in0=xhc_s[:], in1=r1[:], op=ALU.mult)
            tt = pp_s.tile([P, nihc], F32, tag="tt")
            nc.vector.tensor_tensor(out=tt[:], in0=thc[:], in1=tlc[:], op=ALU.add)
            u1 = pp_s.tile([P, nihc], F32, tag="u1")
            nc.vector.scalar_tensor_tensor(
                out=u1[:], in0=tt[:], scalar=2.0, in1=r2[:], op0=ALU.mult, op1=ALU.add
            )
            bhalfc = pp_s.tile([P, nihc], F32, tag="bhalfc")
            nc.vector.tensor_scalar(
                out=bhalfc[:],
                in0=u1[:],
                scalar1=snegc[:, 0:1],
                scalar2=None,
                op0=ALU.add,
            )

            # ---- exchange B halves within the batch pair ----
            nc.sync.dma_start(
                out=bh_dram.rearrange("a (t p) -> p (a t)", p=P), in_=bhalfc[:]
            )
            if single:
                nc.sync.dma_start(out=bfull_dram[0:1, :], in_=bh_dram)
            else:
                nc.gpsimd.collective_compute(
                    "AllGather",
                    ALU.bypass,
                    replica_groups=groups,
                    ins=[bh_dram],
                    outs=[bfull_dram],
                )

            bflat = bfull_dram.rearrange("a b -> (a b)")
            nc.sync.dma_start(
                out=bcol[:], in_=bflat.rearrange("(t p) -> p t", p=P)
            )
            nc.vector.tensor_scalar_mul(nbcol[:], bcol[:], -1.0)

            # B splits in column layout -> DRAM (row order) -> stack rows
            bsh = pp_s.tile([P, nic], BF16, tag="bsh")
            bsm = pp_s.tile([P, nic], BF16, tag="bsm")
            bsl = pp_s.tile([P, nic], BF16, tag="bsl")
            brt = pp_s.tile([P, nic], F32, tag="brt")
            brt2 = pp_s.tile([P, nic], F32, tag="brt2")
            nc.vector.tensor_copy(out=bsh[:], in_=bcol[:])
            nc.vector.tensor_tensor(out=brt[:], in0=bcol[:], in1=bsh[:], op=ALU.subtract)
            nc.vector.tensor_copy(out=bsm[:], in_=brt[:])
            nc.vector.tensor_tensor(
                out=brt2[:], in0=brt[:], in1=bsm[:], op=ALU.subtract
            )
            nc.vector.tensor_copy(out=bsl[:], in_=brt2[:])
            for row, t_ in ((0, bsh), (1, bsm), (2, bsl)):
                nc.sync.dma_start(
                    out=bspl_dram[row].rearrange("(t p) -> p t", p=P), in_=t_[:]
                )

            # stats rhs source stacks per i-half: rows [Bh Bm Bl xh xm xl]
            src_s = [
                pp_s.tile([6, nih], BF16, tag=f"src_s{h}", name=f"src_s{h}")
                for h in range(nhalves)
            ]
            for h in range(nhalves):
                hs = slice(h * nih, (h + 1) * nih)
                nc.sync.dma_start(out=src_s[h][0:1, :], in_=bspl_dram[0:1, hs])
                nc.sync.dma_start(out=src_s[h][1:2, :], in_=bspl_dram[1:2, hs])
                nc.sync.dma_start(out=src_s[h][2:3, :], in_=bspl_dram[2:3, hs])
                nc.sync.dma_start(out=src_s[h][3:4, :], in_=xh[0:1, hs])
                nc.sync.dma_start(out=src_s[h][4:5, :], in_=xm[0:1, hs])
                nc.sync.dma_start(out=src_s[h][5:6, :], in_=xl[0:1, hs])

            # stats lhsT stack [9, nj] and rhs stacks [9, nih] per half
            src_l = pp_s.tile([3, nj], BF16, tag="src_l")
            nc.sync.dma_start(out=src_l[0:1, :], in_=chi)
            nc.sync.dma_start(out=src_l[1:2, :], in_=clo)
            nc.sync.dma_start(out=src_l[2:3, :], in_=ones_row[0:1, 0:nj])
            with tc.tile_pool(name="stk", bufs=1, space="PSUM") as stk:
                p9 = stk.tile([9, max(nj, nih)], F32)
                mm512(p9[:, 0:nj], sel_l_s_s[:], src_l[:])
                nc.scalar.copy(out=l9[:], in_=p9[:, 0:nj])
                for h in range(nhalves):
                    mm512(p9[:, 0:nih], sel_r_s_s[:], src_s[h][:])
                    nc.scalar.copy(out=r9[h][:], in_=p9[:, 0:nih])

            # output lhsT stack [9, n] from [xh; xm; xl; ones]
            src_lo = pp_s.tile([4, n], BF16, tag="src_lo")
            nc.sync.dma_start(out=src_lo[0:1, :], in_=xh)
            nc.sync.dma_start(out=src_lo[1:2, :], in_=xm)
            nc.sync.dma_start(out=src_lo[2:3, :], in_=xl)
            nc.sync.dma_start(out=src_lo[3:4, :], in_=ones_row)
            with tc.tile_pool(name="stko", bufs=1, space="PSUM") as stko:
                for hh in range(2):
                    p9o = stko.tile([9, n // 2], F32)
                    mm512(
                        p9o[:],
                        sel_l_o_s[:],
                        src_lo[:, hh * (n // 2) : (hh + 1) * (n // 2)],
                    )
                    nc.scalar.copy(
                        out=l9o[:, hh * (n // 2) : (hh + 1) * (n // 2)], in_=p9o[:]
                    )

        # ---------------- Phases S+O interleaved over j groups ----------------
        nc.sync.dma_start(out=src_o[3:4, :], in_=chi)
        nc.sync.dma_start(out=src_o[4:5, :], in_=clo)

        spool = ctx.enter_context(tc.tile_pool(name="sz", bufs=2, space="PSUM"))
        opool = ctx.enter_context(tc.tile_pool(name="oz", bufs=2, space="PSUM"))
        qtp = ctx.enter_context(tc.tile_pool(name="qtp", bufs=1, space="PSUM"))
        epool = ctx.enter_context(tc.tile_pool(name="escr", bufs=2))
        mpool = ctx.enter_context(tc.tile_pool(name="m", bufs=8))
        qspl = ctx.enter_context(tc.tile_pool(name="qspl", bufs=2))
        outp = ctx.enter_context(tc.tile_pool(name="outp", bufs=2))

        for g in range(n_jg):
            gs = g * jcs_per_g
            for jci in range(jcs_per_g):
                jc = gs + jci
                lhs = l9[:, jc * P : (jc + 1) * P]
                mq = mpool.tile([P, niq], F32, tag="mq")
                for q in range(niq):
                    h, qq = divmod(q * iq, nih)
                    zp = spool.tile([P, iq], F32, tag="sz")
                    mm512(zp[:], lhs, r9[h][:, qq : qq + iq])
                    nc.vector.tensor_reduce(
                        out=mq[:, q : q + 1],
                        in_=zp[:],
                        axis=mybir.AxisListType.X,
                        op=ALU.max,
                    )
                m = mpool.tile([P, 1], F32, tag="m")
                nc.vector.tensor_reduce(
                    out=m[:], in_=mq[:], axis=mybir.AxisListType.X, op=ALU.max
                )
                nm = mpool.tile([P, 1], F32, tag="nm")
                nc.vector.tensor_scalar_mul(nm[:], m[:], -1.0)
                dq = mpool.tile([P, niq], F32, tag="dq")
                for q in range(niq):
                    h, qq = divmod(q * iq, nih)
                    zp = spool.tile([P, iq], F32, tag="sz")
                    mm512(zp[:], lhs, r9[h][:, qq : qq + iq])
                    e = epool.tile([P, iq], F32, tag="e")
                    nc.scalar.activation(
                        out=e[:],
                        in_=zp[:],
                        func=AF.Exp,
                        bias=nm[0:P, 0:1],
                        scale=1.0,
                        accum_out=dq[:, q : q + 1],
                    )
                d = mpool.tile([P, 1], F32, tag="d")
                nc.vector.tensor_reduce(
                    out=d[:], in_=dq[:], axis=mybir.AxisListType.X, op=ALU.add
                )
                lnd = mpool.tile([P, 1], F32, tag="lnd")
                nc.scalar.activation(out=lnd[:], in_=d[:], func=AF.Ln)
                nc.vector.tensor_tensor(
                    out=qcol[:, jc : jc + 1], in0=m[:], in1=lnd[:], op=ALU.add
                )

            # Q splits for this group's columns, interleaved [P, jcs, 3],
            # then one transpose-matmul per 128-j chunk (out base 0)
            ge = gs + jcs_per_g
            qcs = qspl.tile([P, jcs_per_g, 3], BF16, tag="qcs")
            qt1 = qspl.tile([P, jcs_per_g], F32, tag="qt1")
            qt2 = qspl.tile([P, jcs_per_g], F32, tag="qt2")
            nc.vector.tensor_copy(out=qcs[:, :, 0], in_=qcol[:, gs:ge])
            nc.vector.tensor_tensor(
                out=qt1[:], in0=qcol[:, gs:ge], in1=qcs[:, :, 0], op=ALU.subtract
            )
            nc.vector.tensor_copy(out=qcs[:, :, 1], in_=qt1[:])
            nc.vector.tensor_tensor(
                out=qt2[:], in0=qt1[:], in1=qcs[:, :, 1], op=ALU.subtract
            )
            nc.vector.tensor_copy(out=qcs[:, :, 2], in_=qt2[:])

            qsp = qtp.tile([3, jg], F32, tag="qsp")
            for jci in range(jcs_per_g):
                nc.tensor.matmul(
                    qsp[:, jci * P : (jci + 1) * P],
                    qcs[:, jci, :],
                    i128_s[:],
                    start=True,
                    stop=True,
                )
            nc.scalar.copy(out=src_o[0:3, g * jg : (g + 1) * jg], in_=qsp[:])
            q9p = qtp.tile([9, jg], F32, tag="q9p")
            mm512(q9p[:], sel_r_o_s[:], src_o[:, g * jg : (g + 1) * jg])
            nc.scalar.copy(out=r9o[:, g * jg : (g + 1) * jg], in_=q9p[:])

            # ---- output pass for this j group ----
            for ich in range(nic // icg):
                ot = outp.tile([P, icg, jg], F32, tag="ot")
                for ici in range(icg):
                    ic = ich * icg + ici
                    ozp = opool.tile([P, jg], F32, tag="oz")
                    nc.tensor.matmul(
                        ozp[:],
                        l9o[:, ic * P : (ic + 1) * P],
                        r9o[:, g * jg : (g + 1) * jg],
                        start=True,
                        stop=True,
                    )
                    nc.scalar.activation(
                        out=ot[:, ici, :],
                        in_=ozp[:],
                        func=AF.Exp,
                        bias=nbcol[0:P, ic : ic + 1],
                        scale=1.0,
                    )
                nc.sync.dma_start(
                    out=out.rearrange("(ic p) j -> p ic j", p=P)[
                        :, ich * icg : (ich + 1) * icg, g * jg : (g + 1) * jg
                    ],
                    in_=ot[:],
                )

    nc.compile()
    return nc


# ---------------------------------------------------------------------------


def make_in_maps(scores, n, mode="pair"):
    """Per-core input dicts. Core c -> batch c//2, halves h = c%2."""
    single = mode == "single"
    nj = n if single else n // 2
    nih = n if single else n // 2
    nkc = n // P
    ncores = 1 if single else N_CORES

    cfull = (2 * np.arange(n) + 1 - n).astype(np.float32)
    ch_f, cl_f = _split2(cfull)

    common = {
        "i128": np.eye(P, dtype=ml_dtypes.bfloat16),
        "sel_l_s": _bf(SEL_L_S),
        "sel_r_s": _bf(SEL_R_S),
        "sel_l_o": _bf(SEL_L_O),
        "sel_r_o": _bf(SEL_R_O),
        "ones3": np.ones((3, P), dtype=ml_dtypes.bfloat16),
        "onesc": np.ones((P, 1), dtype=ml_dtypes.bfloat16),
        "ones_row": np.ones((1, n), dtype=ml_dtypes.bfloat16),
    }

    in_maps = []
    for c in range(ncores):
        b = 0 if single else c // 2
        h = 0 if single else c % 2
        x = np.asarray(scores[b], dtype=np.float32)
        xh_, xm_, xl_ = _split3(x)
        xch, xcl = _split2(x)
        xcol = np.ascontiguousarray(x.reshape(nkc, P).T)
        xchc = np.ascontiguousarray(xch.reshape(nkc, P).T)
        xclc = np.ascontiguousarray(xcl.reshape(nkc, P).T)
        blhs = np.zeros((P, 3 * nkc), dtype=ml_dtypes.bfloat16)
        blhs[:, 0::3] = 1.0
        blhs[:, 1::3] = xchc
        blhs[:, 2::3] = xclc
        xc2 = np.concatenate([xchc, xclc], axis=1)
        sl = slice(h * nih, h * nih + nih)
        sj = slice(h * nj, h * nj + nj)
        in_maps.append(
            {
                "xh": xh_[None, :],
                "xm": xm_[None, :],
                "xl": xl_[None, :],
                "xhh": xh_[None, sl],
                "xmh": xm_[None, sl],
                "xlh": xl_[None, sl],
                "xcol": xcol,
                "xhc": np.ascontiguousarray(x[sl].reshape(-1, P).T),
                "blhs": blhs,
                "xc2": xc2,
                "chi": ch_f[None, sj],
                "clo": cl_f[None, sj],
                **common,
            }
        )
    return in_maps


_NC_CACHE = {}


def _get_nc(n):
    if n not in _NC_CACHE:
        _NC_CACHE[n] = build_nc(n=n, mode="pair", num_devices=N_CORES)
    return _NC_CACHE[n]


def kernel(scores):
    scores = np.asarray(scores, dtype=np.float32)
    b, n = scores.shape
    nj = n // 2
    nc = _get_nc(n)
    in_maps = make_in_maps(scores, n, mode="pair")
    res = run_bass_kernel_spmd(nc, in_maps, list(range(N_CORES)))
    out = np.empty((b, n, n), dtype=np.float32)
    for c in range(N_CORES):
        bb, h = c // 2, c % 2
        out[bb, :, h * nj : (h + 1) * nj] = res.results[c]["out"]
    return out


# revision 58
# speedup vs baseline: 204.3456x; 204.3456x over previous
"""NeuralSort relaxed-permutation kernel for 8 Trainium2 NeuronCores.

out[b, i, j] = softmax_i( s_i * scaling_j - B_i ),  s = -scores[b]
  scaling_j = n - 1 - 2j   =>  z[i,j] = c_j * x_i - B_i  with x = scores[b],
  c_j = -(n - 1 - 2j) = 2j + 1 - n
  B_i = sum_k |x_i - x_k| = x_i*(n - 2*cnt_i) - S + 2*t_i
        cnt_i = #{k: x_k > x_i},  t_i = sum_{k: x_k > x_i} x_k,  S = sum_k x_k

Sharding: core c -> (batch b = c//2, j-half h = c%2). Each core emits the
full-i (n) by half-j (n/2) slab of batch b.

Per-core pipeline (all matmuls bf16 @ 1 cyc/row, exact via hi/mid/lo splits):
  P: comparison tiles split across DVE (is_lt -> {0,1}) and ACT (Sign ->
     {-1,0,1}); PE reduces them with [ones|x_hi|x_lo] stationaries into
     cnt/t rows; combine to the B half in column layout; AllGather [B; r1b]
     within the batch pair (r1b = #below - #above, a signed rank).
  M-bound: i's are bucketed into 128 rank ranges by r1b (mask tiles + PE
     mask-matmul -> per-bucket mean (x_bar, B_bar)). z evaluated at the 128
     bucket means underestimates each column max by <~40 (z is flat near
     its optimum in rank space; B_bar >= f(x_bar) by convexity of
     f(x) = sum_k|x - x_k|), which is all the exp shift needs.
  S: per 128-j chunk: M'_j = rowmax of l9-slice^T @ rep9 (one tiny matmul +
     one [128,128] DVE reduce); z = c_j x_i - B_i via K=9 bf16 stacked
     matmul into PSUM (j on partitions, i on free); ACT exp(z - M') with
     accum_out -> D_j; Q_j = M'_j + ln(D_j) (Ln batched per j group).
  O: z' = c_j x_i - Q_j - B_i via K=12 bf16 matmul (i on partitions, j on
     free; B folded in so exp needs no bias and spans ic-pairs); ACT exp
     from PSUM -> final values in SBUF; one 4 MiB DMA per 16 i-chunks,
     staged per 512-j group so DMA overlaps the next group's stats.
"""

from contextlib import ExitStack

import numpy as np
import ml_dtypes

import concourse.bass as bass
import concourse.tile as tile
from concourse import bacc, mybir
from concourse.bass_utils import run_bass_kernel_spmd

F32 = mybir.dt.float32
BF16 = mybir.dt.bfloat16
AF = mybir.ActivationFunctionType
ALU = mybir.AluOpType

N_CORES = 8
P = 128


def _bf(x):
    return np.asarray(x, dtype=ml_dtypes.bfloat16)


def _split3(x):
    x = np.asarray(x, dtype=np.float32)
    h = _bf(x)
    r = x - h.astype(np.float32)
    m = _bf(r)
    l = _bf(r - m.astype(np.float32))
    return h, m, l


def _split2(x):
    x = np.asarray(x, dtype=np.float32)
    h = _bf(x)
    l = _bf(x - h.astype(np.float32))
    return h, l


# K-row pairing for the z matmuls (z = sum_k lhsT_row_k * rhs_row_k).
# Stacks put PE-transposed rows first (matmul outputs must land at partition
# base 0); DMA-filled rows follow (DMA can target any partition).
# Stats: lhsT rows from [chi, clo, ones]; rhs rows from [Bh,Bm,Bl,xh,xm,xl].
SEL_L_S = np.zeros((3, 9), np.float32)
SEL_R_S = np.zeros((6, 9), np.float32)
_PAIRS_S = [
    (0, 3, 1.0),   # c_hi * x_h
    (2, 0, -1.0),  # 1 * -B_h
    (1, 3, 1.0),   # c_lo * x_h
    (0, 4, 1.0),   # c_hi * x_m
    (2, 1, -1.0),  # 1 * -B_m
    (1, 4, 1.0),   # c_lo * x_m
    (0, 5, 1.0),   # c_hi * x_l
    (2, 2, -1.0),  # 1 * -B_l
    (1, 5, 1.0),   # c_lo * x_l
]
for k, (ls, rs, w) in enumerate(_PAIRS_S):
    SEL_L_S[ls, k] = 1.0
    SEL_R_S[rs, k] = w

# Output: z'' = c_j x_i - Q_j - B_i, K=12. lhsT rows from
# [xh, xm, xl, ones, Bh, Bm, Bl]; rhs rows from [Qh, Qm, Ql, chi, clo, ones].
# Folding B into the matmul removes the ACT bias, letting exp span ic-pairs.
SEL_L_O = np.zeros((7, 12), np.float32)
SEL_R_O = np.zeros((6, 12), np.float32)
_PAIRS_O = [
    (0, 3, 1.0),   # x_h * c_hi
    (3, 0, -1.0),  # 1 * -Q_h
    (4, 5, -1.0),  # B_h * -1
    (0, 4, 1.0),   # x_h * c_lo
    (1, 3, 1.0),   # x_m * c_hi
    (3, 1, -1.0),  # 1 * -Q_m
    (5, 5, -1.0),  # B_m * -1
    (1, 4, 1.0),   # x_m * c_lo
    (2, 3, 1.0),   # x_l * c_hi
    (3, 2, -1.0),  # 1 * -Q_l
    (6, 5, -1.0),  # B_l * -1
    (2, 4, 1.0),   # x_l * c_lo
]
for k, (ls, rs, w) in enumerate(_PAIRS_O):
    SEL_L_O[ls, k] = 1.0
    SEL_R_O[rs, k] = w


def _dve_ks(nkc):
    """Comparison chunks assigned to DVE (rest go to ACT as Sign)."""
    if nkc >= 8:
        return [k for k in range(nkc) if k % 8 < 5]
    return [k for k in range(nkc) if k % 2 == 0]


def build_nc(n=4096, mode="pair", num_devices=N_CORES):
    """mode: "pair" (8-core, AllGather B halves); "single" (1-core debug:
    full j/i ranges, no collective); "timing" (pair shapes, collective
    replaced by local row copies -- for the single-core timeline model)."""
    single = mode == "single"
    use_collective = mode == "pair"
    nj = n if single else n // 2    # output columns per core
    nih = n if single else n // 2   # i-range whose B this core computes
    nkc = n // P                    # k-chunks in the G pass
    njc = nj // P                   # 128-wide j-chunks for stats
    nic = n // P                    # 128-wide i-chunks for output
    jg = 512                        # output j staging group width
    n_jg = nj // jg
    jcs_per_g = jg // P
    icg = min(16, nic)              # i-chunks per output DMA
    niq = max(2, n // 1024)         # stats i-quarter count
    iq = n // niq                   # stats free-dim per z tile (<=1024)

    nc = bacc.Bacc(
        "TRN2", target_bir_lowering=False, debug=False, num_devices=num_devices
    )

    def din(name, shape, dt=F32):
        return nc.dram_tensor(name, shape, dt, kind="ExternalInput").ap()

    n_dve = len(_dve_ks(nkc))  # comparison chunks on DVE; rest ACT (Sign)

    xs4 = din("xs4", [4, n], BF16)        # rows [xh; xm; xl; ones]
    xsh3 = din("xsh3", [3, nih], BF16)    # x splits of this core's i-half
    xso3 = din("xso3", [3, nih], BF16)    # x splits of the partner's i-half
    cs3 = din("cs3", [3, nj], BF16)       # rows [chi; clo; ones]
    xcol = din("xcol", [P, nkc], F32)
    xhc = din("xhc", [P, nih // P], F32)  # x of this core's i-half, col layout
    blhs = din("blhs", [P, 3 * nkc], BF16)
    xc2d = din("xc2d", [P, 2 * n_dve], BF16)  # [xch | xcl], DVE-chunk cols
    i128 = din("i128", [P, P], BF16)
    lob = din("lob", [P, P], F32)    # rank-bucket lower bounds along free
    hib = din("hib", [P, P], F32)    # rank-bucket upper bounds along free
    xball = din("xball", [P, nic, 5], BF16)  # chunk cols [xch, xcl, 0, 0, 1]
    sel_l_s = din("sel_l_s", [3, 9], BF16)
    sel_r_s = din("sel_r_s", [6, 9], BF16)
    sel_l_o = din("sel_l_o", [7, 12], BF16)
    sel_r_o = din("sel_r_o", [6, 12], BF16)
    ones3 = din("ones3", [3, P], BF16)
    onesc = din("onesc", [P, 1], BF16)

    out = nc.dram_tensor("out", [n, nj], F32, kind="ExternalOutput").ap()

    # exchange payload rows: [B-half; r1b-half] (r1b = signed-rank surrogate)
    bh_dram = nc.dram_tensor("bh_dram", [2, nih], F32).ap()
    nhalves = 1 if single else 2
    bfull_dram = nc.dram_tensor("bfull_dram", [2 * nhalves, nih], F32).ap()
    b3_dram = nc.dram_tensor("b3_dram", [6, nih], F32).ap()
    bspl_dram = nc.dram_tensor("bspl_dram", [3, n], BF16).ap()
    bsplh_dram = nc.dram_tensor("bsplh_dram", [3, nih], BF16).ap()
    bsplo_dram = nc.dram_tensor("bsplo_dram", [3, nih], BF16).ap()
    groups = [[2 * p, 2 * p + 1] for p in range(max(1, num_devices // 2))]

    def mm512(out_ap, lhsT, rhs, start=True, stop=True):
        """matmul with the moving dim split into <=512-column chunks."""
        nfree = rhs.shape[-1]
        assert out_ap.shape[-1] == nfree
        for o in range(0, nfree, 512):
            e = min(o + 512, nfree)
            nc.tensor.matmul(
                out_ap[..., o:e], lhsT, rhs[..., o:e], start=start, stop=stop
            )

    with tile.TileContext(nc) as tc, ExitStack() as ctx:
        cpool = ctx.enter_context(tc.tile_pool(name="consts", bufs=1))

        def load(pool, ap_dram, shape, dt, name):
            t = pool.tile(shape, dt, tag=name)
            nc.sync.dma_start(out=t[:], in_=ap_dram)
            return t

        # long-lived constants
        i128_s = load(cpool, i128, [P, P], BF16, "i128")
        sel_r_o_s = load(cpool, sel_r_o, [6, 12], BF16, "sel_r_o")
        bcol = cpool.tile([P, nic], F32, tag="bcol")
        l9 = cpool.tile([9, nj], BF16, tag="l9")
        r9 = [
            cpool.tile([9, nih], BF16, tag=f"r9_{h}", name=f"r9_{h}")
            for h in range(nhalves)
        ]
        l9o = cpool.tile([12, n], BF16, tag="l9o")
        r9o = cpool.tile([12, nj], BF16, tag="r9o")
        src_o = cpool.tile([6, nj], BF16, tag="src_o")
        qcol = cpool.tile([P, njc], F32, tag="qcol")
        rep9 = cpool.tile([9, P], BF16, tag="rep9")

        with tc.tile_pool(name="prep", bufs=1) as pp_s:
            xcol_s = load(pp_s, xcol, [P, nkc], F32, "xcol")
            xhc_s = load(pp_s, xhc, [P, nih // P], F32, "xhc")
            blhs_s = load(pp_s, blhs, [P, 3 * nkc], BF16, "blhs")
            xc2d_s = load(pp_s, xc2d, [P, 2 * n_dve], BF16, "xc2d")
            sel_l_s_s = load(pp_s, sel_l_s, [3, 9], BF16, "sel_l_s")
            sel_r_s_s = load(pp_s, sel_r_s, [6, 9], BF16, "sel_r_s")
            sel_l_o_s = load(pp_s, sel_l_o, [7, 12], BF16, "sel_l_o")
            ones3_s = load(pp_s, ones3, [3, P], BF16, "ones3")
            onesc_s = load(pp_s, onesc, [P, 1], BF16, "onesc")
            xsh = load(pp_s, xsh3, [3, nih], BF16, "xsh")

            # ---- Phase P: B over this core's i-half ----
            xb = pp_s.tile([P, nih], F32, tag="xb")
            with tc.tile_pool(name="pp", bufs=1, space="PSUM") as pp:
                xbp = pp.tile([P, nih], F32)
                mm512(xbp[:], ones3_s[:], xsh[:])
                nc.vector.tensor_copy(out=xb[:], in_=xbp[:])

            # comparison pass interleaved between DVE (is_lt -> G in {0,1})
            # and ACT (Sign -> sgn in {-1,0,1}); interleaving keeps both
            # engines fed since PE drains g tiles in program order. For the
            # DVE set, sum_k |x_i-x_k| = x_i*(nD - 2*cntD) - SD + 2*tD; for
            # the ACT set it's x_i*sgnS - tS. Tie terms vanish either way.
            dve_ks = set(_dve_ks(nkc))
            nxcol = pp_s.tile([P, nkc], F32, tag="nxcol")
            nc.vector.tensor_scalar_mul(nxcol[:], xcol_s[:], -1.0)
            b3 = pp_s.tile([3, nih], F32, tag="b3")
            b3s = pp_s.tile([3, nih], F32, tag="b3s")
            with (
                tc.tile_pool(name="bp", bufs=1, space="PSUM") as bp,
                tc.tile_pool(name="gp", bufs=4) as gp,
            ):
                bpsum = bp.tile([3, nih], F32)
                bpsum2 = bp.tile([3, nih], F32)
                ndve_seen = nact_seen = 0
                for k in range(nkc):
                    g = gp.tile([P, nih], BF16, tag="g")
                    if k in dve_ks:
                        ndve_seen += 1
                        nc.vector.tensor_scalar(
                            out=g[:],
                            in0=xb[:],
                            scalar1=xcol_s[:, k : k + 1],
                            scalar2=None,
                            op0=ALU.is_lt,
                        )
                        mm512(
                            bpsum[:],
                            blhs_s[:, 3 * k : 3 * k + 3],
                            g[:],
                            start=(ndve_seen == 1),
                            stop=(ndve_seen == n_dve),
                        )
                    else:
                        nact_seen += 1
                        nc.scalar.activation(
                            out=g[:],
                            in_=xb[:],
                            func=AF.Sign,
                            bias=nxcol[0:P, k : k + 1],
                        )
                        mm512(
                            bpsum2[:],
                            blhs_s[:, 3 * k : 3 * k + 3],
                            g[:],
                            start=(nact_seen == 1),
                            stop=(nact_seen == nkc - n_dve),
                        )
                nc.vector.tensor_copy(out=b3[:], in_=bpsum[:])
                nc.vector.tensor_copy(out=b3s[:], in_=bpsum2[:])

            # roundtrip the six rows through DRAM into column layout (one
            # readback DMA for all six)
            nihc = nih // P
            nc.sync.dma_start(out=b3_dram[0:3, :], in_=b3[:])
            nc.sync.dma_start(out=b3_dram[3:6, :], in_=b3s[:])
            bc_all = pp_s.tile([P, 6 * nihc], F32, tag="bc_all")
            nc.sync.dma_start(
                out=bc_all[:],
                in_=b3_dram.rearrange("r (t p) -> p (r t)", p=P),
            )
            cntc = bc_all[:, 0 * nihc : 1 * nihc]
            thc = bc_all[:, 1 * nihc : 2 * nihc]
            tlc = bc_all[:, 2 * nihc : 3 * nihc]
            sgnc = bc_all[:, 3 * nihc : 4 * nihc]
            tshc = bc_all[:, 4 * nihc : 5 * nihc]
            tslc = bc_all[:, 5 * nihc : 6 * nihc]

            # -SD (sum of x over the DVE-chunk k's)
            sneg = pp_s.tile([1, 1], F32, tag="sneg")
            with tc.tile_pool(name="sp", bufs=1, space="PSUM") as sp:
                sxp = sp.tile([1, 2 * n_dve], F32)
                nc.tensor.matmul(sxp[:], onesc_s[:], xc2d_s[:], start=True, stop=True)
                ssum = pp_s.tile([1, 1], F32, tag="ssum")
                nc.vector.tensor_reduce(
                    out=ssum[:], in_=sxp[:], axis=mybir.AxisListType.X, op=ALU.add
                )
                nc.vector.tensor_scalar_mul(sneg[:], ssum[:], -1.0)
            snegc = pp_s.tile([P, 1], F32, tag="snegc")
            nc.gpsimd.partition_broadcast(snegc[:], sneg[0:1, 0:1])

            # B = x*(nD - 2*cntD + sgnS) - SD + 2*(tDh+tDl) - (tSh+tSl)
            nD = float(n_dve * P)
            r1 = pp_s.tile([P, nihc], F32, tag="r1")
            nc.vector.tensor_scalar(
                out=r1[:],
                in0=cntc[:],
                scalar1=-2.0,
                scalar2=nD,
                op0=ALU.mult,
                op1=ALU.add,
            )
            r1b = pp_s.tile([P, nihc], F32, tag="r1b")
            nc.vector.tensor_tensor(out=r1b[:], in0=r1[:], in1=sgnc[:], op=ALU.add)
            r2 = pp_s.tile([P, nihc], F32, tag="r2")
            nc.vector.tensor_tensor(out=r2[:], in0=xhc_s[:], in1=r1b[:], op=ALU.mult)
            tt = pp_s.tile([P, nihc], F32, tag="tt")
            nc.vector.tensor_tensor(out=tt[:], in0=thc[:], in1=tlc[:], op=ALU.add)
            u1 = pp_s.tile([P, nihc], F32, tag="u1")
            nc.vector.scalar_tensor_tensor(
                out=u1[:], in0=tt[:], scalar=2.0, in1=r2[:], op0=ALU.mult, op1=ALU.add
            )
            tts = pp_s.tile([P, nihc], F32, tag="tts")
            nc.vector.tensor_tensor(out=tts[:], in0=tshc[:], in1=tslc[:], op=ALU.add)
            u2 = pp_s.tile([P, nihc], F32, tag="u2")
            nc.vector.tensor_tensor(out=u2[:], in0=u1[:], in1=tts[:], op=ALU.subtract)
            bhalfc = pp_s.tile([P, nihc], F32, tag="bhalfc")
            nc.vector.tensor_scalar(
                out=bhalfc[:],
                in0=u2[:],
                scalar1=snegc[:, 0:1],
                scalar2=None,
                op0=ALU.add,
            )

            # ---- own-half B splits: available without the exchange, so the
            # own-half stats start while the AllGather is in flight ----
            def col_splits(src_col, dst_tile, w, tg):
                s0 = dst_tile[:, 0 * w : 1 * w]
                s1 = dst_tile[:, 1 * w : 2 * w]
                s2 = dst_tile[:, 2 * w : 3 * w]
                t1 = pp_s.tile([P, w], F32, tag=f"{tg}_t1", name=f"{tg}_t1")
                t2 = pp_s.tile([P, w], F32, tag=f"{tg}_t2", name=f"{tg}_t2")
                nc.vector.tensor_copy(out=s0, in_=src_col)
                nc.vector.tensor_tensor(
                    out=t1[:], in0=src_col, in1=s0, op=ALU.subtract
                )
                nc.vector.tensor_copy(out=s1, in_=t1[:])
                nc.vector.tensor_tensor(
                    out=t2[:], in0=t1[:], in1=s1, op=ALU.subtract
                )
                nc.vector.tensor_copy(out=s2, in_=t2[:])

            bsh_all = pp_s.tile([P, 3 * nihc], BF16, tag="bsh_all")
            col_splits(bhalfc[:], bsh_all, nihc, "so")
            nc.sync.dma_start(
                out=bsplh_dram.rearrange("s (t p) -> p (s t)", p=P), in_=bsh_all[:]
            )

            # ---- exchange [B; r1b] halves within the batch pair ----
            nc.sync.dma_start(
                out=bh_dram[0:1, :].rearrange("a (t p) -> p (a t)", p=P),
                in_=bhalfc[:],
            )
            nc.sync.dma_start(
                out=bh_dram[1:2, :].rearrange("a (t p) -> p (a t)", p=P),
                in_=r1b[:],
            )
            if use_collective:
                nc.gpsimd.collective_compute(
                    "AllGather",
                    ALU.bypass,
                    replica_groups=groups,
                    ins=[bh_dram],
                    outs=[bfull_dram],
                )
            else:
                for hh in range(nhalves):
                    nc.sync.dma_start(
                        out=bfull_dram[2 * hh : 2 * hh + 2, :], in_=bh_dram
                    )

            # column-layout readback of the full B and r1b (true i-order)
            rcol = pp_s.tile([P, nic], F32, tag="rcol")
            for hh in range(nhalves):
                hsl = slice(hh * nihc, (hh + 1) * nihc)
                nc.sync.dma_start(
                    out=bcol[:, hsl],
                    in_=bfull_dram[2 * hh].rearrange("(t p) -> p t", p=P),
                )
                nc.sync.dma_start(
                    out=rcol[:, hsl],
                    in_=bfull_dram[2 * hh + 1].rearrange("(t p) -> p t", p=P),
                )

            # other-half B, position-free: other = (half0 + half1) - own
            if nhalves == 2:
                otherc = pp_s.tile([P, nihc], F32, tag="otherc")
                nc.vector.tensor_tensor(
                    out=otherc[:],
                    in0=bcol[:, 0:nihc],
                    in1=bcol[:, nihc : 2 * nihc],
                    op=ALU.add,
                )
                nc.vector.tensor_tensor(
                    out=otherc[:], in0=otherc[:], in1=bhalfc[:], op=ALU.subtract
                )
                bso_all = pp_s.tile([P, 3 * nihc], BF16, tag="bso_all")
                col_splits(otherc[:], bso_all, nihc, "oo")
                nc.sync.dma_start(
                    out=bsplo_dram.rearrange("s (t p) -> p (s t)", p=P),
                    in_=bso_all[:],
                )
            bs_all = pp_s.tile([P, 3 * nic], BF16, tag="bs_all")
            col_splits(bcol[:], bs_all, nic, "sf")
            nc.sync.dma_start(
                out=bspl_dram.rearrange("s (t p) -> p (s t)", p=P), in_=bs_all[:]
            )

            # ---- rank-bucket representatives for the column-max bound ----
            # Bucket i's by r1b into 128 rank ranges; per-bucket mean point
            # (x_bar, B_bar) lies within ~40 of the true column max for every
            # j whose optimum falls in that bucket (z is flat near its
            # optimum in rank space, and B_bar >= f(x_bar) by convexity).
            lob_s = load(pp_s, lob, [P, P], F32, "lob")
            hib_s = load(pp_s, hib, [P, P], F32, "hib")
            xball_s = load(pp_s, xball, [P, nic, 5], BF16, "xball")
            nc.vector.tensor_copy(
                out=xball_s[:, :, 2], in_=bs_all[:, 0:nic]
            )
            nc.vector.tensor_copy(
                out=xball_s[:, :, 3], in_=bs_all[:, nic : 2 * nic]
            )
            with (
                tc.tile_pool(name="repp", bufs=1, space="PSUM") as repp,
                tc.tile_pool(name="mkp", bufs=4) as mkp,
            ):
                reps = repp.tile([P, 5], F32)
                for ch in range(nic):
                    m1 = mkp.tile([P, P], BF16, tag="m1")
                    nc.vector.tensor_scalar(
                        out=m1[:],
                        in0=lob_s[:],
                        scalar1=rcol[:, ch : ch + 1],
                        scalar2=None,
                        op0=ALU.is_le,
                    )
                    msk = mkp.tile([P, P], BF16, tag="msk")
                    nc.vector.scalar_tensor_tensor(
                        out=msk[:],
                        in0=hib_s[:],
                        scalar=rcol[:, ch : ch + 1],
                        in1=m1[:],
                        op0=ALU.is_gt,
                        op1=ALU.mult,
                    )
                    nc.tensor.matmul(
                        reps[:],
                        msk[:],
                        xball_s[:, ch, :],
                        start=(ch == 0),
                        stop=(ch == nic - 1),
                    )
                # reps rows: [sum xh, sum xl, sum Bh, sum Bm, count]
                repss = pp_s.tile([P, 5], F32, tag="repss")
                nc.vector.tensor_copy(out=repss[:], in_=reps[:])
                reps = repss
                cnt1 = pp_s.tile([P, 1], F32, tag="cnt1")
                nc.vector.tensor_scalar_max(cnt1[:], reps[:, 4:5], 1.0)
                rc = pp_s.tile([P, 1], F32, tag="rc")
                nc.vector.reciprocal(rc[:], cnt1[:])
                repx = pp_s.tile([P, 1], F32, tag="repx")
                nc.vector.tensor_tensor(
                    out=repx[:], in0=reps[:, 0:1], in1=reps[:, 1:2], op=ALU.add
                )
                nc.vector.tensor_tensor(
                    out=repx[:], in0=repx[:], in1=rc[:], op=ALU.mult
                )
                repb = pp_s.tile([P, 1], F32, tag="repb")
                nc.vector.tensor_tensor(
                    out=repb[:], in0=reps[:, 2:3], in1=reps[:, 3:4], op=ALU.add
                )
                nc.vector.tensor_tensor(
                    out=repb[:], in0=repb[:], in1=rc[:], op=ALU.mult
                )
                # empty bucket -> push its line to -inf via a huge B
                iz = pp_s.tile([P, 1], F32, tag="iz")
                nc.vector.tensor_scalar(
                    out=iz[:], in0=reps[:, 4:5], scalar1=0.5, scalar2=None,
                    op0=ALU.is_le,
                )
                nc.vector.scalar_tensor_tensor(
                    out=repb[:], in0=iz[:], scalar=1e30, in1=repb[:],
                    op0=ALU.mult, op1=ALU.add,
                )
                # split cols [Bh2 Bm2 Bl0 xh2 xl2 x0] matching sel_r_s order
                rs6 = pp_s.tile([P, 6], BF16, tag="rs6")
                rtmp = pp_s.tile([P, 1], F32, tag="rep_rt")
                nc.vector.tensor_copy(out=rs6[:, 0:1], in_=repb[:])
                nc.vector.tensor_tensor(
                    out=rtmp[:], in0=repb[:], in1=rs6[:, 0:1], op=ALU.subtract
                )
                nc.vector.tensor_copy(out=rs6[:, 1:2], in_=rtmp[:])
                nc.vector.memset(rs6[:, 2:3], 0.0)
                nc.vector.tensor_copy(out=rs6[:, 3:4], in_=repx[:])
                nc.vector.tensor_tensor(
                    out=rtmp[:], in0=repx[:], in1=rs6[:, 3:4], op=ALU.subtract
                )
                nc.vector.tensor_copy(out=rs6[:, 4:5], in_=rtmp[:])
                nc.vector.memset(rs6[:, 5:6], 0.0)
                # transpose -> [6, 128] -> K=9 rep stack via the stats sel
                p6 = repp.tile([6, P], F32)
                nc.tensor.matmul(p6[:], rs6[:], i128_s[:], start=True, stop=True)
                srep = pp_s.tile([6, P], BF16, tag="srep")
                nc.vector.tensor_copy(out=srep[:], in_=p6[:])
                p9r = repp.tile([9, P], F32)
                nc.tensor.matmul(
                    p9r[:], sel_r_s_s[:], srep[:], start=True, stop=True
                )
                nc.vector.tensor_copy(out=rep9[:], in_=p9r[:])

            # stats rhs source stacks: [0]=own half, [1]=partner half
            # (max/sum over i are order-invariant, so halves need not be in
            # true i-order)
            src_s = [
                pp_s.tile([6, nih], BF16, tag=f"src_s{h}", name=f"src_s{h}")
                for h in range(nhalves)
            ]
            nc.sync.dma_start(out=src_s[0][0:3, :], in_=bsplh_dram)
            nc.sync.dma_start(out=src_s[0][3:6, :], in_=xsh3)
            if nhalves == 2:
                nc.sync.dma_start(out=src_s[1][0:3, :], in_=bsplo_dram)
                nc.sync.dma_start(out=src_s[1][3:6, :], in_=xso3)

            # stats lhsT stack [9, nj] and rhs stacks [9, nih] per half
            src_l = pp_s.tile([3, nj], BF16, tag="src_l")
            nc.sync.dma_start(out=src_l[:], in_=cs3)
            with tc.tile_pool(name="stk", bufs=1, space="PSUM") as stk:
                p9 = stk.tile([9, max(nj, nih)], F32)
                mm512(p9[:, 0:nj], sel_l_s_s[:], src_l[:])
                nc.vector.tensor_copy(out=l9[:], in_=p9[:, 0:nj])
                for h in range(nhalves):
                    mm512(p9[:, 0:nih], sel_r_s_s[:], src_s[h][:])
                    nc.vector.tensor_copy(out=r9[h][:], in_=p9[:, 0:nih])

            # output lhsT stack [9, n] from [xh; xm; xl; ones]
            src_lo = pp_s.tile([7, n], BF16, tag="src_lo")
            nc.sync.dma_start(out=src_lo[0:4, :], in_=xs4)
            nc.sync.dma_start(out=src_lo[4:7, :], in_=bspl_dram)
            with tc.tile_pool(name="stko", bufs=1, space="PSUM") as stko:
                for hh in range(2):
                    p9o = stko.tile([12, n // 2], F32)
                    mm512(
                        p9o[:],
                        sel_l_o_s[:],
                        src_lo[:, hh * (n // 2) : (hh + 1) * (n // 2)],
                    )
                    nc.vector.tensor_copy(
                        out=l9o[:, hh * (n // 2) : (hh + 1) * (n // 2)], in_=p9o[:]
                    )

        # ---------------- Phases S+O interleaved over j groups ----------------
        nc.sync.dma_start(out=src_o[3:6, :], in_=cs3)

        spool = ctx.enter_context(tc.tile_pool(name="sz", bufs=2, space="PSUM"))
        opool = ctx.enter_context(tc.tile_pool(name="oz", bufs=2, space="PSUM"))
        epool = ctx.enter_context(tc.tile_pool(name="escr", bufs=2))
        mpool = ctx.enter_context(tc.tile_pool(name="m", bufs=8))
        qspl = ctx.enter_context(tc.tile_pool(name="qspl", bufs=2))
        outp = ctx.enter_context(tc.tile_pool(name="outp", bufs=2))

        for g in range(n_jg):
            gs = g * jcs_per_g
            mg = mpool.tile([P, jcs_per_g], F32, tag="mg", name="mg")
            dg = mpool.tile([P, jcs_per_g], F32, tag="dg", name="dg")
            for jci in range(jcs_per_g):
                jc = gs + jci
                lhs = l9[:, jc * P : (jc + 1) * P]
                zrp = spool.tile([P, P], F32, tag="sz")
                nc.tensor.matmul(zrp[:], lhs, rep9[:], start=True, stop=True)
                m = mpool.tile([P, 1], F32, tag="m")
                nc.vector.tensor_reduce(
                    out=m[:], in_=zrp[:], axis=mybir.AxisListType.X, op=ALU.max
                )
                nc.vector.tensor_copy(out=mg[:, jci : jci + 1], in_=m[:])
                nm = mpool.tile([P, 1], F32, tag="nm")
                nc.vector.tensor_scalar_mul(nm[:], m[:], -1.0)
                dq = mpool.tile([P, niq], F32, tag="dq")
                for q in range(niq):
                    h, qq = divmod(q * iq, nih)
                    zp = spool.tile([P, iq], F32, tag="sz")
                    mm512(zp[:], lhs, r9[h][:, qq : qq + iq])
                    e = epool.tile([P, iq], F32, tag="e")
                    nc.scalar.activation(
                        out=e[:],
                        in_=zp[:],
                        func=AF.Exp,
                        bias=nm[0:P, 0:1],
                        scale=1.0,
                        accum_out=dq[:, q : q + 1],
                    )
                nc.vector.tensor_reduce(
                    out=dg[:, jci : jci + 1],
                    in_=dq[:],
                    axis=mybir.AxisListType.X,
                    op=ALU.add,
                )
            # one batched Ln per group keeps the ACT Exp table resident longer
            lndg = mpool.tile([P, jcs_per_g], F32, tag="lndg", name="lndg")
            nc.scalar.activation(out=lndg[:], in_=dg[:], func=AF.Ln)
            nc.vector.tensor_tensor(
                out=qcol[:, gs : gs + jcs_per_g], in0=mg[:], in1=lndg[:], op=ALU.add
            )

            # Q splits for this group's columns, interleaved [P, jcs, 3],
            # then one transpose-matmul per 128-j chunk (out base 0)
            ge = gs + jcs_per_g
            qcs = qspl.tile([P, jcs_per_g, 3], BF16, tag="qcs")
            qt1 = qspl.tile([P, jcs_per_g], F32, tag="qt1")
            qt2 = qspl.tile([P, jcs_per_g], F32, tag="qt2")
            nc.vector.tensor_copy(out=qcs[:, :, 0], in_=qcol[:, gs:ge])
            nc.vector.tensor_tensor(
                out=qt1[:], in0=qcol[:, gs:ge], in1=qcs[:, :, 0], op=ALU.subtract
            )
            nc.vector.tensor_copy(out=qcs[:, :, 1], in_=qt1[:])
            nc.vector.tensor_tensor(
                out=qt2[:], in0=qt1[:], in1=qcs[:, :, 1], op=ALU.subtract
            )
            nc.vector.tensor_copy(out=qcs[:, :, 2], in_=qt2[:])

            qsp = opool.tile([3, jg], F32, tag="oz", name="qsp")
            for jci in range(jcs_per_g):
                nc.tensor.matmul(
                    qsp[:, jci * P : (jci + 1) * P],
                    qcs[:, jci, :],
                    i128_s[:],
                    start=True,
                    stop=True,
                )
            nc.vector.tensor_copy(out=src_o[0:3, g * jg : (g + 1) * jg], in_=qsp[:])
            q9p = opool.tile([12, jg], F32, tag="oz", name="q9p")
            mm512(q9p[:], sel_r_o_s[:], src_o[:, g * jg : (g + 1) * jg])
            nc.vector.tensor_copy(out=r9o[:, g * jg : (g + 1) * jg], in_=q9p[:])

            # ---- output pass for this j group ----
            for ich in range(nic // icg):
                ot = outp.tile([P, icg, jg], F32, tag="ot")
                for ici in range(0, icg, 2):
                    ozp = opool.tile([P, 2 * jg], F32, tag="oz")
                    for u in range(2):
                        ic = ich * icg + ici + u
                        nc.tensor.matmul(
                            ozp[:, u * jg : (u + 1) * jg],
                            l9o[:, ic * P : (ic + 1) * P],
                            r9o[:, g * jg : (g + 1) * jg],
                            start=True,
                            stop=True,
                        )
                    nc.scalar.activation(
                        out=ot[:, ici : ici + 2, :],
                        in_=ozp[:],
                        func=AF.Exp,
                    )
                nc.sync.dma_start(
                    out=out.rearrange("(ic p) j -> p ic j", p=P)[
                        :, ich * icg : (ich + 1) * icg, g * jg : (g + 1) * jg
                    ],
                    in_=ot[:],
                )

    nc.compile()
    return nc


# ---------------------------------------------------------------------------


def make_in_maps(scores, n, mode="pair"):
    """Per-core input dicts. Core c -> batch c//2, halves h = c%2."""
    single = mode == "single"
    nj = n if single else n // 2
    nih = n if single else n // 2
    nkc = n // P
    ncores = 1 if single else N_CORES

    cfull = (2 * np.arange(n) + 1 - n).astype(np.float32)
    ch_f, cl_f = _split2(cfull)

    dve_ks = _dve_ks(nkc)
    n_dve = len(dve_ks)

    common = {
        "i128": np.eye(P, dtype=ml_dtypes.bfloat16),
        "sel_l_s": _bf(SEL_L_S),
        "sel_r_s": _bf(SEL_R_S),
        "sel_l_o": _bf(SEL_L_O),
        "sel_r_o": _bf(SEL_R_O),
        "ones3": np.ones((3, P), dtype=ml_dtypes.bfloat16),
        "onesc": np.ones((P, 1), dtype=ml_dtypes.bfloat16),
    }

    in_maps = []
    for c in range(ncores):
        b = 0 if single else c // 2
        h = 0 if single else c % 2
        x = np.asarray(scores[b], dtype=np.float32)
        xh_, xm_, xl_ = _split3(x)
        xch, xcl = _split2(x)
        xcol = np.ascontiguousarray(x.reshape(nkc, P).T)
        xchc = np.ascontiguousarray(xch.reshape(nkc, P).T)
        xclc = np.ascontiguousarray(xcl.reshape(nkc, P).T)
        blhs = np.zeros((P, 3 * nkc), dtype=ml_dtypes.bfloat16)
        blhs[:, 0::3] = 1.0
        blhs[:, 1::3] = xchc
        blhs[:, 2::3] = xclc
        xc2d = np.concatenate([xchc[:, dve_ks], xclc[:, dve_ks]], axis=1)
        assert xc2d.shape[1] == 2 * n_dve
        sl = slice(h * nih, h * nih + nih)
        sj = slice(h * nj, h * nj + nj)
        ones_n = np.ones((1, n), dtype=ml_dtypes.bfloat16)
        xs4 = np.concatenate(
            [xh_[None, :], xm_[None, :], xl_[None, :], ones_n], axis=0
        )
        xsh3 = np.concatenate(
            [xh_[None, sl], xm_[None, sl], xl_[None, sl]], axis=0
        )
        so = slice((1 - h) * nih, (1 - h) * nih + nih) if not single else sl
        xso3 = np.concatenate(
            [xh_[None, so], xm_[None, so], xl_[None, so]], axis=0
        )
        cs3 = np.concatenate(
            [ch_f[None, sj], cl_f[None, sj], np.ones((1, nj), ml_dtypes.bfloat16)],
            axis=0,
        )
        lo_row = (-n + np.arange(P) * (2 * n // P)).astype(np.float32)
        lob = np.tile(lo_row[None, :], (P, 1))
        hib = lob + float(2 * n // P)
        xball = np.zeros((P, n // P, 5), dtype=ml_dtypes.bfloat16)
        xball[:, :, 0] = xchc
        xball[:, :, 1] = xclc
        xball[:, :, 4] = 1.0
        in_maps.append(
            {
                "xs4": xs4,
                "lob": lob,
                "hib": hib,
                "xball": xball,
                "xsh3": xsh3,
                "xso3": xso3,
                "cs3": cs3,
                "xcol": xcol,
                "xhc": np.ascontiguousarray(x[sl].reshape(-1, P).T),
                "blhs": blhs,
                "xc2d": np.ascontiguousarray(xc2d),
                **common,
            }
        )
    return in_maps


_NC_CACHE = {}


def _get_nc(n):
    if n not in _NC_CACHE:
        _NC_CACHE[n] = build_nc(n=n, mode="pair", num_devices=N_CORES)
    return _NC_CACHE[n]


def kernel(scores):
    scores = np.asarray(scores, dtype=np.float32)
    b, n = scores.shape
    nj = n // 2
    nc = _get_nc(n)
    in_maps = make_in_maps(scores, n, mode="pair")
    res = run_bass_kernel_spmd(nc, in_maps, list(range(N_CORES)))
    out = np.empty((b, n, n), dtype=np.float32)
    for c in range(N_CORES):
        bb, h = c // 2, c % 2
        out[bb, :, h * nj : (h + 1) * nj] = res.results[c]["out"]
    return out
